# revision 12
# baseline (speedup 1.0000x reference)
"""DifferentialMultiHeadAttention TRN2 Bass kernel.

Sharding: 2 branches x 16 heads = 32 head-instances, 4 per core (core 0-3:
branch 1, core 4-7: branch 2). Each core computes its heads' attention,
applies its lambda-scaled head-output projection and the full final proj on
its rank-partial y; the host sums the 8 partial outputs (valid because wo,
the lambda-mix and proj are linear) and adds the folded bias vector.

QKV biases are handled exactly by augmenting the contraction dim: x' =
[x, 1, 0..] (D 1024 -> 1152 = 9*128), w' = [w; b; 0..]. All matmuls run as
float32r (TF32-like: full PE rate at N>=256, ~1e-4 rel err measured on HW).

Softmax is computed without max-subtraction (scores are O(5), exp is safe in
fp32) via a transposed layout: scoresT[s,t] tiles feed exp (ScalarE,
PSUM->SBUF), then AV accumulates lhsT=[v|1] so PSUM row 64 is the softmax
denominator; the reciprocal row is broadcast across partitions with a K=1
outer-product matmul and applied with one vector multiply.
"""

import sys

for _p in ("/opt/trn_rl_repo", "/opt/pypackages"):
    if _p not in sys.path:
        sys.path.append(_p)

import numpy as np

DIM, H, HD = 1024, 16, 64
B = 2
DA = 1152          # augmented contraction dim (bias row + pad)
NDT = DA // 128    # 9 d-tiles
NPT = DIM // 128   # 8 d-tiles for proj
CH = 512           # token chunk size
NH = 4             # heads per core
NCORES = 8


def build(S=2048):
    """Build the per-core SPMD Bass program for per-batch seq len S."""
    import concourse.bacc as bacc
    import concourse.bass as bass
    import concourse.mybir as mybir
    import concourse.tile as tile

    f32 = mybir.dt.float32
    f32r = mybir.dt.float32r

    T = B * S                    # total tokens
    NC = S // CH                 # chunks per batch
    NST = S // 128               # s-tiles per batch

    nc = bacc.Bacc("TRN2", target_bir_lowering=False, debug=False,
                   num_devices=NCORES)

    xta = nc.dram_tensor("xta", [DA, T], f32r, kind="ExternalInput")
    wq = nc.dram_tensor("wq", [DA, 256], f32r, kind="ExternalInput")
    wk = nc.dram_tensor("wk", [DA, 256], f32r, kind="ExternalInput")
    wv = nc.dram_tensor("wv", [DA, 256], f32r, kind="ExternalInput")
    wo = nc.dram_tensor("wo", [256, DIM], f32r, kind="ExternalInput")
    pw = nc.dram_tensor("pw", [DIM, DIM], f32r, kind="ExternalInput")
    one = nc.dram_tensor("one", [128, 64], f32r, kind="ExternalInput")
    z = nc.dram_tensor("z", [T, DIM], f32, kind="ExternalOutput")

    with tile.TileContext(nc) as tc:
        with (
            nc.allow_low_precision(
                reason="f32r storage is 4-byte fp32; PSUM accumulation stays fp32"),
            tc.tile_pool(name="consts", bufs=1) as consts,
            tc.tile_pool(name="kv", bufs=1) as kv,
            tc.tile_pool(name="xp", bufs=2) as xp,
            tc.tile_pool(name="work", bufs=4) as work,
            tc.tile_pool(name="outp", bufs=2) as outp,
            tc.tile_pool(name="yb", bufs=1) as yb,
            tc.tile_pool(name="pp", bufs=2, space="PSUM") as pp,
            tc.tile_pool(name="scp", bufs=3, space="PSUM") as scp,
            tc.tile_pool(name="avp", bufs=2, space="PSUM") as avp,
            tc.tile_pool(name="bcy", bufs=1, space="PSUM") as bcy,
        ):
            wq_sb = consts.tile([128, NDT, 256], f32r)
            wk_sb = consts.tile([128, NDT, 256], f32r)
            wv_sb = consts.tile([128, NDT, 256], f32r)
            wo_sb = consts.tile([128, 2, DIM], f32r)
            pw_sb = consts.tile([128, NPT, DIM], f32r)
            ones_sb = consts.tile([1, 64], f32r)

            nc.sync.dma_start(out=wq_sb, in_=wq.ap().rearrange("(dt p) m -> p dt m", p=128))
            nc.sync.dma_start(out=wk_sb, in_=wk.ap().rearrange("(dt p) m -> p dt m", p=128))
            nc.sync.dma_start(out=wv_sb, in_=wv.ap().rearrange("(dt p) m -> p dt m", p=128))
            nc.sync.dma_start(out=wo_sb, in_=wo.ap().rearrange("(pk p) n -> p pk n", p=128))
            nc.sync.dma_start(out=pw_sb, in_=pw.ap().rearrange("(dt p) n -> p dt n", p=128))
            nc.sync.dma_start(out=ones_sb, in_=one.ap()[0:1, 0:64])

            xre = xta.ap().rearrange("(dt p) t -> p dt t", p=128)

            for b in range(B):
                kT = kv.tile([128, 2, S], f32r, tag="kT")
                qT = kv.tile([128, 2, S], f32r, tag="qT")
                va = kv.tile([128, NST, NH, 65], f32r, tag="va")
                nc.sync.dma_start(
                    out=va[:, :, :, 64:65],
                    in_=one.ap()[:, 0:NST * NH].rearrange(
                        "p (st h) -> p st h", st=NST))

                # ---- phase A: Q/K/V generation for this batch ----
                for c in range(NC):
                    tb = b * S + c * CH
                    x_blk = xp.tile([128, NDT, CH], f32r, tag="x")
                    nc.sync.dma_start(out=x_blk, in_=xre[:, :, tb:tb + CH])

                    for wsb, dst in ((wq_sb, qT), (wk_sb, kT)):
                        for pk in range(2):
                            ps = pp.tile([128, CH], f32, tag="pp")
                            for dt_i in range(NDT):
                                nc.tensor.matmul(
                                    ps[:], (wsb[:, dt_i, 128 * pk:128 * pk + 128]),
                                    (x_blk[:, dt_i, :]),
                                    start=(dt_i == 0), stop=(dt_i == NDT - 1))
                            nc.vector.tensor_copy(
                                dst[:, pk, c * CH:(c + 1) * CH], ps[:])

                    for tt in range(4):
                        ps = pp.tile([128, 256], f32, tag="pp")
                        for dt_i in range(NDT):
                            nc.tensor.matmul(
                                ps[:], (x_blk[:, dt_i, 128 * tt:128 * tt + 128]),
                                (wv_sb[:, dt_i, :]),
                                start=(dt_i == 0), stop=(dt_i == NDT - 1))
                        st = c * 4 + tt
                        nc.vector.tensor_copy(
                            va[:, st, :, 0:64],
                            ps.rearrange("p (h d) -> p h d", h=NH))

                # ---- phase B: attention + wo + proj per chunk ----
                for c in range(NC):
                    tb = b * S + c * CH
                    outT = outp.tile([128, 2, CH], f32r, tag="outT")

                    for h in range(NH):
                        pk, row = h // 2, 64 * (h % 2)
                        av = avp.tile([128, CH], f32, tag="av")
                        for st in range(NST):
                            sc = scp.tile([128, CH], f32, tag="sc")
                            nc.tensor.matmul(
                                sc[:],
                                (kT[row:row + 64, pk, st * 128:(st + 1) * 128]),
                                (qT[row:row + 64, pk, c * CH:(c + 1) * CH]),
                                start=True, stop=True)
                            ex = work.tile([128, CH], f32r, tag="ex")
                            nc.scalar.activation(
                                ex[:], sc[:], mybir.ActivationFunctionType.Exp)
                            nc.tensor.matmul(
                                av[0:65, :], (va[:, st, h, :]), (ex[:]),
                                start=(st == 0), stop=(st == NST - 1))

                        rcp = work.tile([1, CH], f32r, tag="rcp")
                        nc.vector.reciprocal(rcp[:], av[64:65, :])
                        bc = bcy.tile([64, CH], f32, tag="bcy")
                        nc.tensor.matmul(bc[:], (ones_sb[:]), (rcp[:]),
                                         start=True, stop=True)
                        u = work.tile([64, CH], f32r, tag="u")
                        nc.vector.tensor_copy(u[:], av[0:64, :])
                        nc.vector.tensor_mul(outT[row:row + 64, pk, :], u[:], bc[:])

                    y_sb = yb.tile([128, NPT, CH], f32r, tag="y")
                    for dto in range(NPT):
                        yp = bcy.tile([128, CH], f32, tag="bcy")
                        for pk in range(2):
                            nc.tensor.matmul(
                                yp[:], (wo_sb[:, pk, dto * 128:(dto + 1) * 128]),
                                (outT[:, pk, :]),
                                start=(pk == 0), stop=(pk == 1))
                        nc.vector.tensor_copy(y_sb[:, dto, :], yp[:])

                    for tt in range(4):
                        for ec in range(2):
                            zp = pp.tile([128, CH], f32, tag="pp")
                            for dt_i in range(NPT):
                                nc.tensor.matmul(
                                    zp[:], (y_sb[:, dt_i, 128 * tt:128 * tt + 128]),
                                    (pw_sb[:, dt_i, ec * CH:(ec + 1) * CH]),
                                    start=(dt_i == 0), stop=(dt_i == NPT - 1))
                            zs = work.tile([128, CH], f32, tag="zs")
                            nc.vector.tensor_copy(zs[:], zp[:])
                            nc.sync.dma_start(
                                out=z.ap()[tb + tt * 128: tb + (tt + 1) * 128,
                                           ec * CH:(ec + 1) * CH],
                                in_=zs[:])

    nc.compile()
    return nc


def get_lambda(lambda_param, layer_idx):
    lf = np.clip(float(np.asarray(layer_idx)) * 0.3, 0.0, 5.0)
    offset = 0.6 * np.exp(-lf)
    lam = (1.0 / (1.0 + np.exp(-float(np.asarray(lambda_param).reshape(-1)[0])))
           ) * (1.0 - offset) + 0.2
    return float(np.clip(lam, 0.1, 0.9))


def prep(inputs, S=2048):
    """Host-side shard prep: returns (in_maps, bias_vec)."""
    x = np.asarray(inputs["x"], np.float32)
    T = B * S
    x2 = np.ascontiguousarray(x.reshape(T, DIM))
    xta = np.zeros((DA, T), np.float32)
    xta[:DIM] = x2.T
    xta[DIM] = 1.0

    lam = get_lambda(inputs["lambda_param"], inputs["layer_idx"])
    pw = np.asarray(inputs["proj_w"], np.float32)

    in_maps = []
    for c in range(NCORES):
        br = c // 4 + 1
        lamf = (1.0 - lam) if br == 1 else lam
        hs = slice(4 * (c % 4), 4 * (c % 4) + 4)

        def aug(w, bias, scale=1.0):
            wa = np.zeros((DA, NH, HD), np.float32)
            wa[:DIM] = np.asarray(w, np.float32)[:, hs]
            wa[DIM] = np.asarray(bias, np.float32)[hs]
            return np.ascontiguousarray((wa * scale).reshape(DA, NH * HD))

        wo_c = np.ascontiguousarray(
            (np.asarray(inputs[f"wo{br}"], np.float32)[hs] * lamf
             ).reshape(256, DIM))
        in_maps.append({
            "one": np.ones((128, 64), np.float32),
            "xta": xta,
            "wq": aug(inputs[f"wq{br}"], inputs[f"bq{br}"], 1.0 / np.sqrt(HD)),
            "wk": aug(inputs[f"wk{br}"], inputs[f"bk{br}"]),
            "wv": aug(inputs[f"wv{br}"], inputs[f"bv{br}"]),
            "wo": wo_c,
            "pw": pw,
        })

    lam32 = np.float32(lam)
    yb = ((1 - lam32) * np.asarray(inputs["bo1"], np.float32)
          + lam32 * np.asarray(inputs["bo2"], np.float32))
    bias_vec = yb.astype(np.float64) @ pw.astype(np.float64) \
        + np.asarray(inputs["proj_b"], np.float64)
    return in_maps, bias_vec


_NC_CACHE = {}


def _get_nc(S=2048):
    if S not in _NC_CACHE:
        _NC_CACHE[S] = build(S)
    return _NC_CACHE[S]


def run(inputs, S=2048, trace=False):
    """Returns (full_output, exec_time_ns_or_None)."""
    from concourse import bass_utils

    nc = _get_nc(S)
    in_maps, bias_vec = prep(inputs, S)
    res = bass_utils.run_bass_kernel_spmd(
        nc, in_maps, core_ids=list(range(NCORES)), trace=trace)
    acc = np.zeros((B * S, DIM), np.float64)
    for c in range(NCORES):
        acc += res.results[c]["z"].astype(np.float64)
    out = (acc + bias_vec).reshape(B, S, DIM).astype(np.float32)
    return out, res.exec_time_ns


def kernel(**inputs):
    out, _ = run(inputs, S=2048, trace=False)
    return out


# revision 15
# speedup vs baseline: 1.4863x; 1.4863x over previous
"""DifferentialMultiHeadAttention TRN2 Bass kernel.

Sharding: 2 branches x 16 heads = 32 head-instances, 4 per core (core 0-3:
branch 1, core 4-7: branch 2). Each core computes its heads' attention,
applies its lambda-scaled head-output projection and the full final proj on
its rank-partial y; the host sums the 8 partial outputs (valid because wo,
the lambda-mix and proj are linear) and adds the folded bias vector.

QKV biases are handled exactly by augmenting the contraction dim: x' =
[x, 1, 0..] (D 1024 -> 1152 = 9*128), w' = [w; b; 0..]. All matmuls run as
float32r (TF32-like: full PE rate at N>=256, ~1e-4 rel err measured on HW).

Softmax is computed without max-subtraction (scores are O(5), exp is safe in
fp32) via a transposed layout: scoresT[s,t] tiles feed exp (ScalarE,
PSUM->SBUF), then AV accumulates lhsT=[v|1] so PSUM row 64 is the softmax
denominator; the reciprocal row is broadcast across partitions with a K=1
outer-product matmul and applied with one vector multiply.
"""

import sys

for _p in ("/opt/trn_rl_repo", "/opt/pypackages"):
    if _p not in sys.path:
        sys.path.append(_p)

import numpy as np
import ml_dtypes


MM_DTYPE = "bf16"   # "bf16" | "f32r"  (matmul operand precision)
MM_NP = ml_dtypes.bfloat16 if MM_DTYPE == "bf16" else np.float32

DIM, H, HD = 1024, 16, 64
B = 2
DA = 1152          # augmented contraction dim (bias row + pad)
NDT = DA // 128    # 9 d-tiles
NPT = DIM // 128   # 8 d-tiles for proj
CH = 512           # token chunk size
NH = 4             # heads per core
NCORES = 8


def build(S=2048):
    """Build the per-core SPMD Bass program for per-batch seq len S."""
    import concourse.bacc as bacc
    import concourse.bass as bass
    import concourse.mybir as mybir
    import concourse.tile as tile

    f32 = mybir.dt.float32
    f32r = {"bf16": mybir.dt.bfloat16, "f32r": mybir.dt.float32r}[MM_DTYPE]

    T = B * S                    # total tokens
    NC = S // CH                 # chunks per batch
    NST = S // 128               # s-tiles per batch

    nc = bacc.Bacc("TRN2", target_bir_lowering=False, debug=False,
                   num_devices=NCORES)

    xta = nc.dram_tensor("xta", [DA, T], f32r, kind="ExternalInput")
    wq = nc.dram_tensor("wq", [DA, 256], f32r, kind="ExternalInput")
    wk = nc.dram_tensor("wk", [DA, 256], f32r, kind="ExternalInput")
    wv = nc.dram_tensor("wv", [DA, 256], f32r, kind="ExternalInput")
    wo = nc.dram_tensor("wo", [256, DIM], f32r, kind="ExternalInput")
    pw = nc.dram_tensor("pw", [DIM, DIM], f32r, kind="ExternalInput")
    one = nc.dram_tensor("one", [128, 64], f32, kind="ExternalInput")
    onem = nc.dram_tensor("onem", [128, 64], f32r, kind="ExternalInput")
    z = nc.dram_tensor("z", [T, DIM], f32, kind="ExternalOutput")

    with tile.TileContext(nc) as tc:
        with (
            nc.allow_low_precision(
                reason="f32r storage is 4-byte fp32; PSUM accumulation stays fp32"),
            tc.tile_pool(name="consts", bufs=1) as consts,
            tc.tile_pool(name="kv", bufs=1) as kv,
            tc.tile_pool(name="xp", bufs=2) as xp,
            tc.tile_pool(name="work", bufs=4) as work,
            tc.tile_pool(name="outp", bufs=2) as outp,
            tc.tile_pool(name="yb", bufs=1) as yb,
            tc.tile_pool(name="pp", bufs=2, space="PSUM") as pp,
            tc.tile_pool(name="scp", bufs=2, space="PSUM") as scp,
            tc.tile_pool(name="avp", bufs=1, space="PSUM") as avp,
            tc.tile_pool(name="bcy", bufs=1, space="PSUM") as bcy,
        ):
            wq_sb = consts.tile([128, NDT, 256], f32r)
            wk_sb = consts.tile([128, NDT, 256], f32r)
            wv_sb = consts.tile([128, NDT, 256], f32r)
            wo_sb = consts.tile([128, 2, DIM], f32r)
            pw_sb = consts.tile([128, NPT, DIM], f32r)
            ones_sb = consts.tile([1, 64], f32)

            nc.sync.dma_start(out=wq_sb, in_=wq.ap().rearrange("(dt p) m -> p dt m", p=128))
            nc.sync.dma_start(out=wk_sb, in_=wk.ap().rearrange("(dt p) m -> p dt m", p=128))
            nc.sync.dma_start(out=wv_sb, in_=wv.ap().rearrange("(dt p) m -> p dt m", p=128))
            nc.sync.dma_start(out=wo_sb, in_=wo.ap().rearrange("(pk p) n -> p pk n", p=128))
            nc.sync.dma_start(out=pw_sb, in_=pw.ap().rearrange("(dt p) n -> p dt n", p=128))
            nc.sync.dma_start(out=ones_sb, in_=one.ap()[0:1, 0:64])

            xre = xta.ap().rearrange("(dt p) t -> p dt t", p=128)

            for b in range(B):
                kT = kv.tile([128, 2, S], f32r, tag="kT")
                qT = kv.tile([128, 2, S], f32r, tag="qT")
                va = kv.tile([128, NST, NH, 65], f32r, tag="va")
                nc.sync.dma_start(
                    out=va[:, :, :, 64:65],
                    in_=onem.ap()[:, 0:NST * NH].rearrange(
                        "p (st h) -> p st h", st=NST))

                # ---- phase A: Q/K/V generation for this batch ----
                for c in range(NC):
                    tb = b * S + c * CH
                    x_blk = xp.tile([128, NDT, CH], f32r, tag="x")
                    nc.sync.dma_start(out=x_blk, in_=xre[:, :, tb:tb + CH])

                    for wsb, dst in ((wq_sb, qT), (wk_sb, kT)):
                        for pk in range(2):
                            ps = pp.tile([128, CH], f32, tag="pp")
                            for dt_i in range(NDT):
                                nc.tensor.matmul(
                                    ps[:], (wsb[:, dt_i, 128 * pk:128 * pk + 128]),
                                    (x_blk[:, dt_i, :]),
                                    start=(dt_i == 0), stop=(dt_i == NDT - 1))
                            nc.vector.tensor_copy(
                                dst[:, pk, c * CH:(c + 1) * CH], ps[:])

                    for tt in range(4):
                        ps = pp.tile([128, 256], f32, tag="pp")
                        for dt_i in range(NDT):
                            nc.tensor.matmul(
                                ps[:], (x_blk[:, dt_i, 128 * tt:128 * tt + 128]),
                                (wv_sb[:, dt_i, :]),
                                start=(dt_i == 0), stop=(dt_i == NDT - 1))
                        st = c * 4 + tt
                        nc.vector.tensor_copy(
                            va[:, st, :, 0:64],
                            ps.rearrange("p (h d) -> p h d", h=NH))

                # ---- phase B: attention + wo + proj per chunk ----
                for c in range(NC):
                    tb = b * S + c * CH
                    outT = outp.tile([128, 2, CH], f32r, tag="outT")

                    for h in range(NH):
                        pk, row = h // 2, 64 * (h % 2)
                        av = avp.tile([128, CH], f32, tag="av")
                        for sp in range(NST // 2):
                            sc = scp.tile([128, 2, CH], f32, tag="sc")
                            for j in range(2):
                                st = 2 * sp + j
                                nc.tensor.matmul(
                                    sc[:, j, :],
                                    (kT[row:row + 64, pk, st * 128:(st + 1) * 128]),
                                    (qT[row:row + 64, pk, c * CH:(c + 1) * CH]),
                                    start=True, stop=True)
                            ex = work.tile([128, 2, CH], f32r, tag="ex")
                            nc.scalar.activation(
                                ex[:], sc[:], mybir.ActivationFunctionType.Exp)
                            for j in range(2):
                                st = 2 * sp + j
                                nc.tensor.matmul(
                                    av[0:65, :], (va[:, st, h, :]), (ex[:, j, :]),
                                    start=(st == 0), stop=(st == NST - 1))

                        den = work.tile([1, CH], f32, tag="den")
                        nc.vector.tensor_copy(den[:], av[64:65, :])
                        rcp = work.tile([1, CH], f32, tag="rcp")
                        nc.vector.reciprocal_approx_fast(rcp[:], den[:])
                        bc = bcy.tile([64, CH], f32, tag="bcy")
                        nc.tensor.matmul(bc[:], (ones_sb[:]), (rcp[:]),
                                         start=True, stop=True)
                        u = work.tile([64, CH], f32r, tag="u")
                        nc.vector.tensor_copy(u[:], av[0:64, :])
                        nc.vector.tensor_mul(outT[row:row + 64, pk, :], u[:], bc[:])

                    y_sb = yb.tile([128, NPT, CH], f32r, tag="y")
                    for dto in range(NPT):
                        yp = bcy.tile([128, CH], f32, tag="bcy")
                        for pk in range(2):
                            nc.tensor.matmul(
                                yp[:], (wo_sb[:, pk, dto * 128:(dto + 1) * 128]),
                                (outT[:, pk, :]),
                                start=(pk == 0), stop=(pk == 1))
                        nc.vector.tensor_copy(y_sb[:, dto, :], yp[:])

                    for tt in range(4):
                        for ec in range(2):
                            zp = pp.tile([128, CH], f32, tag="pp")
                            for dt_i in range(NPT):
                                nc.tensor.matmul(
                                    zp[:], (y_sb[:, dt_i, 128 * tt:128 * tt + 128]),
                                    (pw_sb[:, dt_i, ec * CH:(ec + 1) * CH]),
                                    start=(dt_i == 0), stop=(dt_i == NPT - 1))
                            zs = work.tile([128, CH], f32, tag="zs")
                            nc.vector.tensor_copy(zs[:], zp[:])
                            nc.sync.dma_start(
                                out=z.ap()[tb + tt * 128: tb + (tt + 1) * 128,
                                           ec * CH:(ec + 1) * CH],
                                in_=zs[:])

    nc.compile()
    return nc


def get_lambda(lambda_param, layer_idx):
    lf = np.clip(float(np.asarray(layer_idx)) * 0.3, 0.0, 5.0)
    offset = 0.6 * np.exp(-lf)
    lam = (1.0 / (1.0 + np.exp(-float(np.asarray(lambda_param).reshape(-1)[0])))
           ) * (1.0 - offset) + 0.2
    return float(np.clip(lam, 0.1, 0.9))


def prep(inputs, S=2048):
    """Host-side shard prep: returns (in_maps, bias_vec)."""
    x = np.asarray(inputs["x"], np.float32)
    T = B * S
    x2 = np.ascontiguousarray(x.reshape(T, DIM))
    xta = np.zeros((DA, T), np.float32)
    xta[:DIM] = x2.T
    xta[DIM] = 1.0

    lam = get_lambda(inputs["lambda_param"], inputs["layer_idx"])
    pw = np.asarray(inputs["proj_w"], np.float32)
    xta_mm = xta.astype(MM_NP)
    pw_mm = pw.astype(MM_NP)

    in_maps = []
    for c in range(NCORES):
        br = c // 4 + 1
        lamf = (1.0 - lam) if br == 1 else lam
        hs = slice(4 * (c % 4), 4 * (c % 4) + 4)

        def aug(w, bias, scale=1.0):
            wa = np.zeros((DA, NH, HD), np.float32)
            wa[:DIM] = np.asarray(w, np.float32)[:, hs]
            wa[DIM] = np.asarray(bias, np.float32)[hs]
            return np.ascontiguousarray(
                (wa * scale).reshape(DA, NH * HD)).astype(MM_NP)

        wo_c = np.ascontiguousarray(
            (np.asarray(inputs[f"wo{br}"], np.float32)[hs] * lamf
             ).reshape(256, DIM))
        in_maps.append({
            "one": np.ones((128, 64), np.float32),
            "onem": np.ones((128, 64), MM_NP),
            "xta": xta_mm,
            "wq": aug(inputs[f"wq{br}"], inputs[f"bq{br}"], 1.0 / np.sqrt(HD)),
            "wk": aug(inputs[f"wk{br}"], inputs[f"bk{br}"]),
            "wv": aug(inputs[f"wv{br}"], inputs[f"bv{br}"]),
            "wo": wo_c.astype(MM_NP),
            "pw": pw_mm,
        })

    lam32 = np.float32(lam)
    yb = ((1 - lam32) * np.asarray(inputs["bo1"], np.float32)
          + lam32 * np.asarray(inputs["bo2"], np.float32))
    bias_vec = yb.astype(np.float64) @ pw.astype(np.float64) \
        + np.asarray(inputs["proj_b"], np.float64)
    return in_maps, bias_vec


_NC_CACHE = {}


def _get_nc(S=2048):
    if S not in _NC_CACHE:
        _NC_CACHE[S] = build(S)
    return _NC_CACHE[S]


def run(inputs, S=2048, trace=False):
    """Returns (full_output, exec_time_ns_or_None)."""
    from concourse import bass_utils

    nc = _get_nc(S)
    in_maps, bias_vec = prep(inputs, S)
    res = bass_utils.run_bass_kernel_spmd(
        nc, in_maps, core_ids=list(range(NCORES)), trace=trace)
    acc = np.zeros((B * S, DIM), np.float64)
    for c in range(NCORES):
        acc += res.results[c]["z"].astype(np.float64)
    out = (acc + bias_vec).reshape(B, S, DIM).astype(np.float32)
    return out, res.exec_time_ns


def kernel(**inputs):
    out, _ = run(inputs, S=2048, trace=False)
    return out


# revision 17
# speedup vs baseline: 1.4975x; 1.0076x over previous
"""DifferentialMultiHeadAttention TRN2 Bass kernel.

Sharding: 2 branches x 16 heads = 32 head-instances, 4 per core (core 0-3:
branch 1, core 4-7: branch 2). Each core computes its heads' attention,
applies its lambda-scaled head-output projection and the full final proj on
its rank-partial y; the host sums the 8 partial outputs (valid because wo,
the lambda-mix and proj are linear) and adds the folded bias vector.

QKV biases are handled exactly by augmenting the contraction dim: x' =
[x, 1, 0..] (D 1024 -> 1152 = 9*128), w' = [w; b; 0..]. All matmuls run as
float32r (TF32-like: full PE rate at N>=256, ~1e-4 rel err measured on HW).

Softmax is computed without max-subtraction (scores are O(5), exp is safe in
fp32) via a transposed layout: scoresT[s,t] tiles feed exp (ScalarE,
PSUM->SBUF), then AV accumulates lhsT=[v|1] so PSUM row 64 is the softmax
denominator; the reciprocal row is broadcast across partitions with a K=1
outer-product matmul and applied with one vector multiply.
"""

import sys

for _p in ("/opt/trn_rl_repo", "/opt/pypackages"):
    if _p not in sys.path:
        sys.path.append(_p)

import numpy as np
import ml_dtypes


MM_DTYPE = "bf16"   # "bf16" | "f32r"  (matmul operand precision)
MM_NP = ml_dtypes.bfloat16 if MM_DTYPE == "bf16" else np.float32

DIM, H, HD = 1024, 16, 64
B = 2
DA = 1152          # augmented contraction dim (bias row + pad)
NDT = DA // 128    # 9 d-tiles
NPT = DIM // 128   # 8 d-tiles for proj
CH = 512           # token chunk size
NH = 4             # heads per core
NCORES = 8


def build(S=2048):
    """Build the per-core SPMD Bass program for per-batch seq len S."""
    import concourse.bacc as bacc
    import concourse.bass as bass
    import concourse.mybir as mybir
    import concourse.tile as tile

    f32 = mybir.dt.float32
    f32r = {"bf16": mybir.dt.bfloat16, "f32r": mybir.dt.float32r}[MM_DTYPE]

    T = B * S                    # total tokens
    NC = S // CH                 # chunks per batch
    NST = S // 128               # s-tiles per batch

    nc = bacc.Bacc("TRN2", target_bir_lowering=False, debug=False,
                   num_devices=NCORES)

    xta = nc.dram_tensor("xta", [DA, T], f32r, kind="ExternalInput")
    wq = nc.dram_tensor("wq", [DA, 256], f32r, kind="ExternalInput")
    wk = nc.dram_tensor("wk", [DA, 256], f32r, kind="ExternalInput")
    wv = nc.dram_tensor("wv", [DA, 256], f32r, kind="ExternalInput")
    wo = nc.dram_tensor("wo", [256, DIM], f32r, kind="ExternalInput")
    pw = nc.dram_tensor("pw", [DIM, DIM], f32r, kind="ExternalInput")
    one = nc.dram_tensor("one", [128, 64], f32, kind="ExternalInput")
    onem = nc.dram_tensor("onem", [128, 64], f32r, kind="ExternalInput")
    z = nc.dram_tensor("z", [T, DIM], f32, kind="ExternalOutput")

    with tile.TileContext(nc) as tc:
        with (
            nc.allow_low_precision(
                reason="f32r storage is 4-byte fp32; PSUM accumulation stays fp32"),
            tc.tile_pool(name="consts", bufs=1) as consts,
            tc.tile_pool(name="kv", bufs=1) as kv,
            tc.tile_pool(name="xp", bufs=2) as xp,
            tc.tile_pool(name="work", bufs=4) as work,
            tc.tile_pool(name="outp", bufs=2) as outp,
            tc.tile_pool(name="yb", bufs=1) as yb,
            tc.tile_pool(name="scp", bufs=2, space="PSUM") as scp,
            tc.tile_pool(name="flx", bufs=4, space="PSUM") as flx,
        ):
            wq_sb = consts.tile([128, NDT, 256], f32r)
            wk_sb = consts.tile([128, NDT, 256], f32r)
            wv_sb = consts.tile([128, NDT, 256], f32r)
            wo_sb = consts.tile([128, 2, DIM], f32r)
            pw_sb = consts.tile([128, NPT, DIM], f32r)
            ones_sb = consts.tile([1, 64], f32)

            nc.sync.dma_start(out=wq_sb, in_=wq.ap().rearrange("(dt p) m -> p dt m", p=128))
            nc.sync.dma_start(out=wk_sb, in_=wk.ap().rearrange("(dt p) m -> p dt m", p=128))
            nc.sync.dma_start(out=wv_sb, in_=wv.ap().rearrange("(dt p) m -> p dt m", p=128))
            nc.sync.dma_start(out=wo_sb, in_=wo.ap().rearrange("(pk p) n -> p pk n", p=128))
            nc.sync.dma_start(out=pw_sb, in_=pw.ap().rearrange("(dt p) n -> p dt n", p=128))
            nc.sync.dma_start(out=ones_sb, in_=one.ap()[0:1, 0:64])

            xre = xta.ap().rearrange("(dt p) t -> p dt t", p=128)

            for b in range(B):
                kT = kv.tile([128, 2, S], f32r, tag="kT")
                qT = kv.tile([128, 2, S], f32r, tag="qT")
                va = kv.tile([128, NST, NH, 65], f32r, tag="va")
                nc.sync.dma_start(
                    out=va[:, :, :, 64:65],
                    in_=onem.ap()[:, 0:NST * NH].rearrange(
                        "p (st h) -> p st h", st=NST))

                # ---- phase A: Q/K/V generation for this batch ----
                for c in range(NC):
                    tb = b * S + c * CH
                    x_blk = xp.tile([128, NDT, CH], f32r, tag="x")
                    nc.sync.dma_start(out=x_blk, in_=xre[:, :, tb:tb + CH])

                    for wsb, dst in ((wq_sb, qT), (wk_sb, kT)):
                        for pk in range(2):
                            ps = flx.tile([128, CH], f32, tag="flex")
                            for dt_i in range(NDT):
                                nc.tensor.matmul(
                                    ps[:], (wsb[:, dt_i, 128 * pk:128 * pk + 128]),
                                    (x_blk[:, dt_i, :]),
                                    start=(dt_i == 0), stop=(dt_i == NDT - 1))
                            nc.vector.tensor_copy(
                                dst[:, pk, c * CH:(c + 1) * CH], ps[:])

                    for tt in range(4):
                        ps = flx.tile([128, 256], f32, tag="flex")
                        for dt_i in range(NDT):
                            nc.tensor.matmul(
                                ps[:], (x_blk[:, dt_i, 128 * tt:128 * tt + 128]),
                                (wv_sb[:, dt_i, :]),
                                start=(dt_i == 0), stop=(dt_i == NDT - 1))
                        st = c * 4 + tt
                        nc.vector.tensor_copy(
                            va[:, st, :, 0:64],
                            ps.rearrange("p (h d) -> p h d", h=NH))

                # ---- phase B: attention + wo + proj per chunk ----
                for c in range(NC):
                    tb = b * S + c * CH
                    outT = outp.tile([128, 2, CH], f32r, tag="outT")

                    for pk in range(2):
                        # head pair (2*pk, 2*pk+1): score matmuls interleave
                        # rows 0-63 / 64-127 so they run concurrently on the
                        # PE's disjoint row-groups.
                        avs = [flx.tile([128, CH], f32, tag="flex",
                                        name=f"av{pk}_{i}")
                               for i in range(2)]
                        for sp in range(NST // 2):
                            scs = [scp.tile([128, 2, CH], f32, tag="sc",
                                            name=f"sc{pk}_{sp}_{i}")
                                   for i in range(2)]
                            for j in range(2):
                                st = 2 * sp + j
                                for hh in range(2):
                                    row = 64 * hh
                                    nc.tensor.matmul(
                                        scs[hh][:, j, :],
                                        (kT[row:row + 64, pk, st * 128:(st + 1) * 128]),
                                        (qT[row:row + 64, pk, c * CH:(c + 1) * CH]),
                                        start=True, stop=True)
                            exs = []
                            for hh in range(2):
                                ex = work.tile([128, 2, CH], f32r, tag="ex")
                                nc.scalar.activation(
                                    ex[:], scs[hh][:],
                                    mybir.ActivationFunctionType.Exp)
                                exs.append(ex)
                            for j in range(2):
                                st = 2 * sp + j
                                for hh in range(2):
                                    h = 2 * pk + hh
                                    nc.tensor.matmul(
                                        avs[hh][0:65, :], (va[:, st, h, :]),
                                        (exs[hh][:, j, :]),
                                        start=(st == 0), stop=(st == NST - 1))

                        for hh in range(2):
                            av, row = avs[hh], 64 * hh
                            den = work.tile([1, CH], f32, tag="den")
                            nc.vector.tensor_copy(den[:], av[64:65, :])
                            rcp = work.tile([1, CH], f32, tag="rcp")
                            nc.vector.reciprocal_approx_fast(rcp[:], den[:])
                            bc = flx.tile([64, CH], f32, tag="flex")
                            nc.tensor.matmul(bc[:], (ones_sb[:]), (rcp[:]),
                                             start=True, stop=True)
                            u = work.tile([64, CH], f32r, tag="u")
                            nc.vector.tensor_copy(u[:], av[0:64, :])
                            nc.vector.tensor_mul(
                                outT[row:row + 64, pk, :], u[:], bc[:])

                    y_sb = yb.tile([128, NPT, CH], f32r, tag="y")
                    for dto in range(NPT):
                        yp = flx.tile([128, CH], f32, tag="flex")
                        for pk in range(2):
                            nc.tensor.matmul(
                                yp[:], (wo_sb[:, pk, dto * 128:(dto + 1) * 128]),
                                (outT[:, pk, :]),
                                start=(pk == 0), stop=(pk == 1))
                        nc.vector.tensor_copy(y_sb[:, dto, :], yp[:])

                    for tt in range(4):
                        for ec in range(2):
                            zp = flx.tile([128, CH], f32, tag="flex")
                            for dt_i in range(NPT):
                                nc.tensor.matmul(
                                    zp[:], (y_sb[:, dt_i, 128 * tt:128 * tt + 128]),
                                    (pw_sb[:, dt_i, ec * CH:(ec + 1) * CH]),
                                    start=(dt_i == 0), stop=(dt_i == NPT - 1))
                            zs = work.tile([128, CH], f32, tag="zs")
                            nc.vector.tensor_copy(zs[:], zp[:])
                            nc.sync.dma_start(
                                out=z.ap()[tb + tt * 128: tb + (tt + 1) * 128,
                                           ec * CH:(ec + 1) * CH],
                                in_=zs[:])

    nc.compile()
    return nc


def get_lambda(lambda_param, layer_idx):
    lf = np.clip(float(np.asarray(layer_idx)) * 0.3, 0.0, 5.0)
    offset = 0.6 * np.exp(-lf)
    lam = (1.0 / (1.0 + np.exp(-float(np.asarray(lambda_param).reshape(-1)[0])))
           ) * (1.0 - offset) + 0.2
    return float(np.clip(lam, 0.1, 0.9))


def prep(inputs, S=2048):
    """Host-side shard prep: returns (in_maps, bias_vec)."""
    x = np.asarray(inputs["x"], np.float32)
    T = B * S
    x2 = np.ascontiguousarray(x.reshape(T, DIM))
    xta = np.zeros((DA, T), np.float32)
    xta[:DIM] = x2.T
    xta[DIM] = 1.0

    lam = get_lambda(inputs["lambda_param"], inputs["layer_idx"])
    pw = np.asarray(inputs["proj_w"], np.float32)
    xta_mm = xta.astype(MM_NP)
    pw_mm = pw.astype(MM_NP)

    in_maps = []
    for c in range(NCORES):
        br = c // 4 + 1
        lamf = (1.0 - lam) if br == 1 else lam
        hs = slice(4 * (c % 4), 4 * (c % 4) + 4)

        def aug(w, bias, scale=1.0):
            wa = np.zeros((DA, NH, HD), np.float32)
            wa[:DIM] = np.asarray(w, np.float32)[:, hs]
            wa[DIM] = np.asarray(bias, np.float32)[hs]
            return np.ascontiguousarray(
                (wa * scale).reshape(DA, NH * HD)).astype(MM_NP)

        wo_c = np.ascontiguousarray(
            (np.asarray(inputs[f"wo{br}"], np.float32)[hs] * lamf
             ).reshape(256, DIM))
        in_maps.append({
            "one": np.ones((128, 64), np.float32),
            "onem": np.ones((128, 64), MM_NP),
            "xta": xta_mm,
            "wq": aug(inputs[f"wq{br}"], inputs[f"bq{br}"], 1.0 / np.sqrt(HD)),
            "wk": aug(inputs[f"wk{br}"], inputs[f"bk{br}"]),
            "wv": aug(inputs[f"wv{br}"], inputs[f"bv{br}"]),
            "wo": wo_c.astype(MM_NP),
            "pw": pw_mm,
        })

    lam32 = np.float32(lam)
    yb = ((1 - lam32) * np.asarray(inputs["bo1"], np.float32)
          + lam32 * np.asarray(inputs["bo2"], np.float32))
    bias_vec = yb.astype(np.float64) @ pw.astype(np.float64) \
        + np.asarray(inputs["proj_b"], np.float64)
    return in_maps, bias_vec


_NC_CACHE = {}


def _get_nc(S=2048):
    if S not in _NC_CACHE:
        _NC_CACHE[S] = build(S)
    return _NC_CACHE[S]


def run(inputs, S=2048, trace=False):
    """Returns (full_output, exec_time_ns_or_None)."""
    from concourse import bass_utils

    nc = _get_nc(S)
    in_maps, bias_vec = prep(inputs, S)
    res = bass_utils.run_bass_kernel_spmd(
        nc, in_maps, core_ids=list(range(NCORES)), trace=trace)
    acc = np.zeros((B * S, DIM), np.float64)
    for c in range(NCORES):
        acc += res.results[c]["z"].astype(np.float64)
    out = (acc + bias_vec).reshape(B, S, DIM).astype(np.float32)
    return out, res.exec_time_ns


def kernel(**inputs):
    out, _ = run(inputs, S=2048, trace=False)
    return out


# revision 18
# speedup vs baseline: 1.5465x; 1.0327x over previous
"""DifferentialMultiHeadAttention TRN2 Bass kernel.

Sharding: 2 branches x 16 heads = 32 head-instances, 4 per core (core 0-3:
branch 1, core 4-7: branch 2). Each core computes its heads' attention,
applies its lambda-scaled head-output projection and the full final proj on
its rank-partial y; the host sums the 8 partial outputs (valid because wo,
the lambda-mix and proj are linear) and adds the folded bias vector.

QKV biases are handled exactly by augmenting the contraction dim: x' =
[x, 1, 0..] (D 1024 -> 1152 = 9*128), w' = [w; b; 0..]. All matmuls run as
float32r (TF32-like: full PE rate at N>=256, ~1e-4 rel err measured on HW).

Softmax is computed without max-subtraction (scores are O(5), exp is safe in
fp32) via a transposed layout: scoresT[s,t] tiles feed exp (ScalarE,
PSUM->SBUF), then AV accumulates lhsT=[v|1] so PSUM row 64 is the softmax
denominator; the reciprocal row is broadcast across partitions with a K=1
outer-product matmul and applied with one vector multiply.
"""

import sys

for _p in ("/opt/trn_rl_repo", "/opt/pypackages"):
    if _p not in sys.path:
        sys.path.append(_p)

import numpy as np
import ml_dtypes


MM_DTYPE = "bf16"   # "bf16" | "f32r"  (matmul operand precision)
MM_NP = ml_dtypes.bfloat16 if MM_DTYPE == "bf16" else np.float32

DIM, H, HD = 1024, 16, 64
B = 2
DA = 1152          # augmented contraction dim (bias row + pad)
NDT = DA // 128    # 9 d-tiles
NPT = DIM // 128   # 8 d-tiles for proj
CH = 512           # token chunk size
NH = 4             # heads per core
NCORES = 8


def build(S=2048):
    """Build the per-core SPMD Bass program for per-batch seq len S."""
    import concourse.bacc as bacc
    import concourse.bass as bass
    import concourse.mybir as mybir
    import concourse.tile as tile

    f32 = mybir.dt.float32
    f32r = {"bf16": mybir.dt.bfloat16, "f32r": mybir.dt.float32r}[MM_DTYPE]

    T = B * S                    # total tokens
    NC = S // CH                 # chunks per batch
    NST = S // 128               # s-tiles per batch

    nc = bacc.Bacc("TRN2", target_bir_lowering=False, debug=False,
                   num_devices=NCORES)

    xta = nc.dram_tensor("xta", [DA, T], f32r, kind="ExternalInput")
    wq = nc.dram_tensor("wq", [DA, 256], f32r, kind="ExternalInput")
    wk = nc.dram_tensor("wk", [DA, 256], f32r, kind="ExternalInput")
    wv = nc.dram_tensor("wv", [DA, 256], f32r, kind="ExternalInput")
    wo = nc.dram_tensor("wo", [256, DIM], f32r, kind="ExternalInput")
    pw = nc.dram_tensor("pw", [DIM, DIM], f32r, kind="ExternalInput")
    one = nc.dram_tensor("one", [128, 64], f32, kind="ExternalInput")
    onem = nc.dram_tensor("onem", [128, 64], f32r, kind="ExternalInput")
    z = nc.dram_tensor("z", [T, DIM], f32, kind="ExternalOutput")

    with tile.TileContext(nc) as tc:
        with (
            nc.allow_low_precision(
                reason="f32r storage is 4-byte fp32; PSUM accumulation stays fp32"),
            tc.tile_pool(name="consts", bufs=1) as consts,
            tc.tile_pool(name="kv", bufs=1) as kv,
            tc.tile_pool(name="xp", bufs=2) as xp,
            tc.tile_pool(name="work", bufs=4) as work,
            tc.tile_pool(name="outp", bufs=2) as outp,
            tc.tile_pool(name="yb", bufs=1) as yb,
            tc.tile_pool(name="scp", bufs=2, space="PSUM") as scp,
            tc.tile_pool(name="flx", bufs=4, space="PSUM") as flx,
        ):
            wq_sb = consts.tile([128, NDT, 256], f32r)
            wk_sb = consts.tile([128, NDT, 256], f32r)
            wv_sb = consts.tile([128, NDT, 256], f32r)
            wo_sb = consts.tile([128, 2, DIM], f32r)
            pw_sb = consts.tile([128, NPT, DIM], f32r)
            ones_sb = consts.tile([1, 64], f32)

            nc.sync.dma_start(out=wq_sb, in_=wq.ap().rearrange("(dt p) m -> p dt m", p=128))
            nc.sync.dma_start(out=wk_sb, in_=wk.ap().rearrange("(dt p) m -> p dt m", p=128))
            nc.sync.dma_start(out=wv_sb, in_=wv.ap().rearrange("(dt p) m -> p dt m", p=128))
            nc.sync.dma_start(out=wo_sb, in_=wo.ap().rearrange("(pk p) n -> p pk n", p=128))
            nc.sync.dma_start(out=pw_sb, in_=pw.ap().rearrange("(dt p) n -> p dt n", p=128))
            nc.sync.dma_start(out=ones_sb, in_=one.ap()[0:1, 0:64])

            xre = xta.ap().rearrange("(dt p) t -> p dt t", p=128)

            pending = None

            def emit_wo_proj(tb, outT):
                y_sb = yb.tile([128, NPT, CH], f32r, tag="y", name=f"y{tb}")
                for dto in range(NPT):
                    yp = flx.tile([128, CH], f32, tag="flex", name=f"yp{tb}_{dto}")
                    for pk in range(2):
                        nc.tensor.matmul(
                            yp[:], (wo_sb[:, pk, dto * 128:(dto + 1) * 128]),
                            (outT[:, pk, :]),
                            start=(pk == 0), stop=(pk == 1))
                    nc.vector.tensor_copy(y_sb[:, dto, :], yp[:])

                for tt in range(4):
                    for ec in range(2):
                        zp = flx.tile([128, CH], f32, tag="flex",
                                      name=f"zp{tb}_{tt}_{ec}")
                        for dt_i in range(NPT):
                            nc.tensor.matmul(
                                zp[:], (y_sb[:, dt_i, 128 * tt:128 * tt + 128]),
                                (pw_sb[:, dt_i, ec * CH:(ec + 1) * CH]),
                                start=(dt_i == 0), stop=(dt_i == NPT - 1))
                        zs = work.tile([128, CH], f32, tag="zs",
                                       name=f"zs{tb}_{tt}_{ec}")
                        nc.vector.tensor_copy(zs[:], zp[:])
                        nc.sync.dma_start(
                            out=z.ap()[tb + tt * 128: tb + (tt + 1) * 128,
                                       ec * CH:(ec + 1) * CH],
                            in_=zs[:])

            for b in range(B):
                kT = kv.tile([128, 2, S], f32r, tag="kT")
                qT = kv.tile([128, 2, S], f32r, tag="qT")
                va = kv.tile([128, NST, NH, 65], f32r, tag="va")
                nc.sync.dma_start(
                    out=va[:, :, :, 64:65],
                    in_=onem.ap()[:, 0:NST * NH].rearrange(
                        "p (st h) -> p st h", st=NST))

                # ---- phase A: Q/K/V generation for this batch ----
                for c in range(NC):
                    tb = b * S + c * CH
                    x_blk = xp.tile([128, NDT, CH], f32r, tag="x")
                    nc.sync.dma_start(out=x_blk, in_=xre[:, :, tb:tb + CH])

                    for wsb, dst in ((wq_sb, qT), (wk_sb, kT)):
                        for pk in range(2):
                            ps = flx.tile([128, CH], f32, tag="flex")
                            for dt_i in range(NDT):
                                nc.tensor.matmul(
                                    ps[:], (wsb[:, dt_i, 128 * pk:128 * pk + 128]),
                                    (x_blk[:, dt_i, :]),
                                    start=(dt_i == 0), stop=(dt_i == NDT - 1))
                            nc.vector.tensor_copy(
                                dst[:, pk, c * CH:(c + 1) * CH], ps[:])

                    for tt in range(4):
                        ps = flx.tile([128, 256], f32, tag="flex")
                        for dt_i in range(NDT):
                            nc.tensor.matmul(
                                ps[:], (x_blk[:, dt_i, 128 * tt:128 * tt + 128]),
                                (wv_sb[:, dt_i, :]),
                                start=(dt_i == 0), stop=(dt_i == NDT - 1))
                        st = c * 4 + tt
                        nc.vector.tensor_copy(
                            va[:, st, :, 0:64],
                            ps.rearrange("p (h d) -> p h d", h=NH))

                # ---- phase B: attention per chunk; wo+proj pipelined one
                # chunk behind so the PE never stalls on the normalization
                # tail or at chunk/batch boundaries ----
                for c in range(NC):
                    tb = b * S + c * CH
                    outT = outp.tile([128, 2, CH], f32r, tag="outT",
                                     name=f"outT{b}_{c}")

                    for pk in range(2):
                        # head pair (2*pk, 2*pk+1): score matmuls interleave
                        # rows 0-63 / 64-127 so they run concurrently on the
                        # PE's disjoint row-groups.
                        avs = [flx.tile([128, CH], f32, tag="flex",
                                        name=f"av{pk}_{i}")
                               for i in range(2)]
                        for sp in range(NST // 2):
                            scs = [scp.tile([128, 2, CH], f32, tag="sc",
                                            name=f"sc{pk}_{sp}_{i}")
                                   for i in range(2)]
                            for j in range(2):
                                st = 2 * sp + j
                                for hh in range(2):
                                    row = 64 * hh
                                    nc.tensor.matmul(
                                        scs[hh][:, j, :],
                                        (kT[row:row + 64, pk, st * 128:(st + 1) * 128]),
                                        (qT[row:row + 64, pk, c * CH:(c + 1) * CH]),
                                        start=True, stop=True)
                            exs = []
                            for hh in range(2):
                                ex = work.tile([128, 2, CH], f32r, tag="ex")
                                nc.scalar.activation(
                                    ex[:], scs[hh][:],
                                    mybir.ActivationFunctionType.Exp)
                                exs.append(ex)
                            for j in range(2):
                                st = 2 * sp + j
                                for hh in range(2):
                                    h = 2 * pk + hh
                                    nc.tensor.matmul(
                                        avs[hh][0:65, :], (va[:, st, h, :]),
                                        (exs[hh][:, j, :]),
                                        start=(st == 0), stop=(st == NST - 1))

                        for hh in range(2):
                            av, row = avs[hh], 64 * hh
                            den = work.tile([1, CH], f32, tag="den")
                            nc.vector.tensor_copy(den[:], av[64:65, :])
                            rcp = work.tile([1, CH], f32, tag="rcp")
                            nc.vector.reciprocal_approx_fast(rcp[:], den[:])
                            bc = flx.tile([64, CH], f32, tag="flex")
                            nc.tensor.matmul(bc[:], (ones_sb[:]), (rcp[:]),
                                             start=True, stop=True)
                            u = work.tile([64, CH], f32r, tag="u")
                            nc.vector.tensor_copy(u[:], av[0:64, :])
                            nc.vector.tensor_mul(
                                outT[row:row + 64, pk, :], u[:], bc[:])

                    if pending is not None:
                        emit_wo_proj(*pending)
                    pending = (tb, outT)

            if pending is not None:
                emit_wo_proj(*pending)

    nc.compile()
    return nc


def get_lambda(lambda_param, layer_idx):
    lf = np.clip(float(np.asarray(layer_idx)) * 0.3, 0.0, 5.0)
    offset = 0.6 * np.exp(-lf)
    lam = (1.0 / (1.0 + np.exp(-float(np.asarray(lambda_param).reshape(-1)[0])))
           ) * (1.0 - offset) + 0.2
    return float(np.clip(lam, 0.1, 0.9))


def prep(inputs, S=2048):
    """Host-side shard prep: returns (in_maps, bias_vec)."""
    x = np.asarray(inputs["x"], np.float32)
    T = B * S
    x2 = np.ascontiguousarray(x.reshape(T, DIM))
    xta = np.zeros((DA, T), np.float32)
    xta[:DIM] = x2.T
    xta[DIM] = 1.0

    lam = get_lambda(inputs["lambda_param"], inputs["layer_idx"])
    pw = np.asarray(inputs["proj_w"], np.float32)
    xta_mm = xta.astype(MM_NP)
    pw_mm = pw.astype(MM_NP)

    in_maps = []
    for c in range(NCORES):
        br = c // 4 + 1
        lamf = (1.0 - lam) if br == 1 else lam
        hs = slice(4 * (c % 4), 4 * (c % 4) + 4)

        def aug(w, bias, scale=1.0):
            wa = np.zeros((DA, NH, HD), np.float32)
            wa[:DIM] = np.asarray(w, np.float32)[:, hs]
            wa[DIM] = np.asarray(bias, np.float32)[hs]
            return np.ascontiguousarray(
                (wa * scale).reshape(DA, NH * HD)).astype(MM_NP)

        wo_c = np.ascontiguousarray(
            (np.asarray(inputs[f"wo{br}"], np.float32)[hs] * lamf
             ).reshape(256, DIM))
        in_maps.append({
            "one": np.ones((128, 64), np.float32),
            "onem": np.ones((128, 64), MM_NP),
            "xta": xta_mm,
            "wq": aug(inputs[f"wq{br}"], inputs[f"bq{br}"], 1.0 / np.sqrt(HD)),
            "wk": aug(inputs[f"wk{br}"], inputs[f"bk{br}"]),
            "wv": aug(inputs[f"wv{br}"], inputs[f"bv{br}"]),
            "wo": wo_c.astype(MM_NP),
            "pw": pw_mm,
        })

    lam32 = np.float32(lam)
    yb = ((1 - lam32) * np.asarray(inputs["bo1"], np.float32)
          + lam32 * np.asarray(inputs["bo2"], np.float32))
    bias_vec = yb.astype(np.float64) @ pw.astype(np.float64) \
        + np.asarray(inputs["proj_b"], np.float64)
    return in_maps, bias_vec


_NC_CACHE = {}


def _get_nc(S=2048):
    if S not in _NC_CACHE:
        _NC_CACHE[S] = build(S)
    return _NC_CACHE[S]


def run(inputs, S=2048, trace=False):
    """Returns (full_output, exec_time_ns_or_None)."""
    from concourse import bass_utils

    nc = _get_nc(S)
    in_maps, bias_vec = prep(inputs, S)
    res = bass_utils.run_bass_kernel_spmd(
        nc, in_maps, core_ids=list(range(NCORES)), trace=trace)
    acc = np.zeros((B * S, DIM), np.float64)
    for c in range(NCORES):
        acc += res.results[c]["z"].astype(np.float64)
    out = (acc + bias_vec).reshape(B, S, DIM).astype(np.float32)
    return out, res.exec_time_ns


def kernel(**inputs):
    out, _ = run(inputs, S=2048, trace=False)
    return out
